# revision 9
# baseline (speedup 1.0000x reference)
"""Trainium2 Bass kernel for ChebyshevLayer.

Math:
    t = tanh(x)                                   [B, IN]
    T_0..T_10 = Chebyshev basis of t
    out = sum_n (T_n @ coeffs[:, :, n]) + x @ base_weight

T_0 == 1 collapses to a bias row bias[o] = sum_i coeffs[i, o, 0], computed
on-device from the streamed c0 block and added to every batch block's PSUM
accumulation group as one K=1 matmul.  The remaining contraction is one
K = 11*1024 matmul per 128-row batch block: K-blocks [T_1..T_10, x].

Precision: everything runs in fp16 (chain values, basis operands, W).
fp16's 10-bit mantissa keeps the end-to-end rel err ~2e-3 while every DVE
op qualifies for the packed 2-byte fast modes.  K-blocks listed in FP8_SET
additionally use fp8(e4m3) hi/lo splitting on both sides
    T ~ T_hi + T_lo,   64*W ~ V_hi + V_lo      (single common scale 2^6)
with the three first-order products computed by DoubleRow matmuls (two
K=128 tiles per instruction at 0.5 cyc/row) in a second PSUM group that is
merged at drain: out = acc_main + 2^-6 * acc_f8.

Layout strategy (host-side, pure data marshalling):
  - x is passed pre-transposed per core as xt[c, p, b] = x[b, c*128+p], so
    the basis is built directly in the [i, b] lhsT layout.  No on-device
    transposes.
  - W is passed chunk-major as ws[c, n, p, o] (n: 10 coeff blocks, bw, c0)
    so each i-chunk arrives as one contiguous DMA and is cast straight to
    operand tiles.

Schedule: the W stream (~70us at DMA roofline) is overlapped by NS
stream-follower blocks whose PSUM accumulation groups stay open across the
whole stream, consuming each W chunk as it lands (backfilling earlier
chunks when a block joins).  Remaining blocks run block-wise afterwards,
the fp16 chain (ACT/DVE) running one block ahead of the PE's matmul group.
Drains go straight from PSUM to DRAM (bias already accumulated).

Sharding over 8 cores: batch x4, out-features x2.
Per core: x [2048, 1024], coeffs [1024, 512, 11], bw [1024, 512]
          -> out [2048, 512].
"""

import numpy as np

import concourse.bass as bass
import concourse.mybir as mybir
import concourse.tile as tile
from concourse import bacc
from concourse.bass_utils import run_bass_kernel_spmd

F32 = mybir.dt.float32
F16 = mybir.dt.float16
FP8 = mybir.dt.float8e4
AF = mybir.ActivationFunctionType
OP = mybir.AluOpType
DR = mybir.MatmulPerfMode.DoubleRow

B, IN, OUT = 8192, 1024, 1024
DEG = 10
MB, MO = 4, 2                  # batch shards x out-feature shards
BC, OC = B // MB, OUT // MO    # per-core: 2048 batch rows, 512 out cols
NBLK = BC // 128               # 16 batch blocks per core
NCH = IN // 128                # 8 contraction chunks per K-block
NKB = DEG + 1                  # 11 operand K-blocks: [T1..T10, x]
NW = NKB + 1                   # 12 streamed W blocks (c0 last, for bias)
NS = 3                         # stream-follower blocks
SQRT2 = float(np.sqrt(2.0))

# K-blocks (by operand index: 0=T1 .. 9=T10, 10=x) computed via fp8 hi/lo
# DoubleRow matmuls in steady-state blocks.  () = pure fp16.
FP8_SET = frozenset()
WSC = 64.0                     # fp8 W-side scale (2^6)

_CACHE = {}
LAST_RESULTS = None  # BassKernelResults of the most recent run (for test.py)


def _build_nc():
    nc = bacc.Bacc(None, target_bir_lowering=False)

    xt_d = nc.dram_tensor("xt", [NCH, 128, BC], F32, kind="ExternalInput")
    ws_d = nc.dram_tensor("ws", [NCH, NW, 128, OC], F32, kind="ExternalInput")
    out_d = nc.dram_tensor("out", [BC, OC], F32, kind="ExternalOutput")

    nf8 = len(FP8_SET)
    with tile.TileContext(nc) as tc:
        with (
            tc.tile_pool(name="wpool", bufs=1) as wpool,
            tc.tile_pool(name="cpool", bufs=1) as cpool,
            tc.tile_pool(name="stage", bufs=2) as spool,
            tc.tile_pool(name="c0p", bufs=2) as c0p,
            tc.tile_pool(name="xp", bufs=2) as xp,
            tc.tile_pool(name="bp", bufs=33) as bp,
            tc.tile_pool(name="tp", bufs=6) as tp_,
            tc.tile_pool(name="op", bufs=3) as op_,
            tc.tile_pool(name="pacc", bufs=6, space=bass.MemorySpace.PSUM) as pacc,
            tc.tile_pool(name="pbias", bufs=1, space=bass.MemorySpace.PSUM) as pbias,
        ):
            ones_col = cpool.tile([128, 1], F16, tag="onescol")
            nc.gpsimd.memset(ones_col[:], 1.0)
            ones_row = cpool.tile([1, 128], F16, tag="onesrow")
            nc.gpsimd.memset(ones_row[:], 1.0)
            bias_h = cpool.tile([1, OC], F16, tag="biash")

            # fp16 W tiles per (n, c); fp8 hi/lo pair tiles per (n, cpair)
            w_tiles = [[None] * NCH for _ in range(NKB)]
            wh_tiles = {n: [None] * (NCH // 2) for n in FP8_SET}
            wl_tiles = {n: [None] * (NCH // 2) for n in FP8_SET}
            c0bs = [None] * NCH

            def fetch_x(j):
                xj = xp.tile([128, NCH, 128], F32, tag="xj", name=f"xj{j}")
                nc.sync.dma_start(
                    xj[:],
                    xt_d.rearrange("c p b -> p c b")[:, :, j * 128:(j + 1) * 128])
                return xj

            def chain(j, xj, f8):
                """fp16 Chebyshev basis for block j in [i, b] layout.
                Returns (bas16, hi, lo): fp16 operand tiles for K-blocks not
                in f8, fp8 hi/lo pairs for those in f8."""
                xv = xj[:].rearrange("p c b -> p (c b)")
                bas16, hi, lo = {}, {}, {}

                def bt(m):
                    return bp.tile([128, IN], F16, tag="bas",
                                   name=f"bas{j}_{m}")

                def tt_(m):
                    return tp_.tile([128, IN], F16, tag="tmp",
                                    name=f"tmp{j}_{m}")

                def h8(m):
                    return bp.tile([128, IN], FP8, tag="bh",
                                   name=f"bh{j}_{m}")

                def l8(m):
                    return bp.tile([128, IN], FP8, tag="bl",
                                   name=f"bl{j}_{m}")

                def split(n, val):
                    """fp8 hi/lo from an fp16 value tile."""
                    h = h8(n)
                    nc.vector.tensor_copy(h[:], val[:])
                    l = l8(n)
                    nc.vector.tensor_tensor(l[:], val[:], h[:], OP.subtract)
                    hi[n], lo[n] = h, l

                def leaf(n, s):
                    """T_2k = s - 1 from the fp16 square tile s."""
                    if n in f8:
                        h = h8(n)
                        nc.vector.tensor_scalar(h[:], s[:], 1.0, None,
                                                OP.subtract)
                        l = l8(n)
                        nc.vector.scalar_tensor_tensor(
                            l[:], s[:], 1.0, h[:], OP.subtract, OP.subtract)
                        hi[n], lo[n] = h, l
                        return None
                    v = bt(n)
                    nc.vector.tensor_scalar(v[:], s[:], 1.0, None, OP.subtract)
                    bas16[n] = v
                    return v

                def odd(n, a, b2, t):
                    """T_odd = a*b2 - t (b2 pre-doubled)."""
                    g = tt_(f"g{n}")
                    nc.vector.tensor_tensor(g[:], a[:], b2[:], OP.mult)
                    v = tt_(f"v{n}") if n in f8 else bt(n)
                    nc.vector.tensor_tensor(v[:], g[:], t[:], OP.subtract)
                    if n in f8:
                        split(n, v)
                        return None
                    bas16[n] = v
                    return v

                t = tt_("t") if 0 in f8 else bt(0)
                nc.scalar.activation(t[:], xv, AF.Tanh)
                if 0 in f8:
                    split(0, t)
                else:
                    bas16[0] = t

                if 10 in f8:
                    h = h8(10)
                    nc.vector.tensor_copy(h[:], xv)
                    l = l8(10)
                    nc.vector.tensor_tensor(l[:], xv, h[:], OP.subtract)
                    hi[10], lo[10] = h, l
                else:
                    xb = bt(10)
                    nc.vector.tensor_copy(xb[:], xv)
                    bas16[10] = xb

                s1 = tt_("s1")
                nc.scalar.activation(s1[:], t[:], AF.Square, scale=SQRT2)
                T2 = leaf(1, s1)
                if T2 is None:
                    T2 = bt("T2v")
                    nc.vector.tensor_scalar(T2[:], s1[:], 1.0, None,
                                            OP.subtract)

                w3 = tt_("w3")
                nc.vector.tensor_scalar(w3[:], T2[:], 2.0, -1.0,
                                        OP.mult, OP.add)
                T3 = tt_("T3v") if 2 in f8 else bt(2)
                nc.vector.tensor_tensor(T3[:], t[:], w3[:], OP.mult)
                if 2 in f8:
                    split(2, T3)
                else:
                    bas16[2] = T3

                s2 = tt_("s2")
                nc.scalar.activation(s2[:], T2[:], AF.Square, scale=SQRT2)
                T4 = leaf(3, s2)
                if T4 is None:
                    T4 = bt("T4v")
                    nc.vector.tensor_scalar(T4[:], s2[:], 1.0, None,
                                            OP.subtract)

                d5 = tt_("d5")
                nc.vector.tensor_scalar(d5[:], T3[:], 2.0, None, OP.mult)
                e5 = tt_("e5")
                nc.vector.tensor_tensor(e5[:], T2[:], d5[:], OP.mult)
                T5 = tt_("T5v") if 4 in f8 else bt(4)
                nc.vector.tensor_tensor(T5[:], e5[:], t[:], OP.subtract)
                if 4 in f8:
                    split(4, T5)
                else:
                    bas16[4] = T5

                s3 = tt_("s3")
                nc.scalar.activation(s3[:], T3[:], AF.Square, scale=SQRT2)
                leaf(5, s3)

                f7 = tt_("f7")
                nc.vector.tensor_scalar(f7[:], T4[:], 2.0, None, OP.mult)
                odd(6, T3, f7, t)

                s4 = tt_("s4")
                nc.scalar.activation(s4[:], T4[:], AF.Square, scale=SQRT2)
                leaf(7, s4)

                h9 = tt_("h9")
                nc.vector.tensor_scalar(h9[:], T5[:], 2.0, None, OP.mult)
                odd(8, T4, h9, t)

                s5 = tt_("s5")
                nc.scalar.activation(s5[:], T5[:], AF.Square, scale=SQRT2)
                leaf(9, s5)

                return bas16, hi, lo

            def mm_chunk(acc, bas16, c, start, stop):
                """fp16 matmuls for all NKB K-blocks at chunk c."""
                for n in range(NKB):
                    nc.tensor.matmul(
                        acc[:], bas16[n][:, c * 128:(c + 1) * 128],
                        w_tiles[n][c][:],
                        start=(start and n == 0),
                        stop=(stop and n == NKB - 1))

            # ---- Stream phase (all-fp16 blocks) ----
            xjs = {j: fetch_x(j) for j in range(NS)}
            bases = {0: chain(0, xjs[0], frozenset())[0]}
            accs = {j: pacc.tile([128, OC], F32, tag="acc", name=f"acc{j}")
                    for j in range(NS)}
            pb = pbias.tile([1, OC], F32, tag="pb")

            for c in range(NCH):
                for q in range(4):
                    st = spool.tile([128, 3, OC], F32, tag="st",
                                    name=f"st{c}_{q}")
                    nc.sync.dma_start(
                        st[:],
                        ws_d[c, 3 * q:3 * q + 3].rearrange("n p f -> p n f"))
                    for i in range(3):
                        n = 3 * q + i
                        if n in FP8_SET:
                            p2, half = divmod(c, 2)
                            if half == 0:
                                wh_tiles[n][p2] = wpool.tile(
                                    [128, 2, OC], FP8, tag="wh", bufs=4 * nf8,
                                    name=f"wh{n}_{p2}")
                                wl_tiles[n][p2] = wpool.tile(
                                    [128, 2, OC], FP8, tag="wl", bufs=4 * nf8,
                                    name=f"wl{n}_{p2}")
                            wh = wh_tiles[n][p2]
                            wl = wl_tiles[n][p2]
                            eng = nc.gpsimd if n < 6 else nc.vector
                            eng.tensor_scalar(wh[:, half, :], st[:, i, :],
                                              WSC, None, OP.mult)
                            nc.vector.scalar_tensor_tensor(
                                wl[:, half, :], st[:, i, :], WSC,
                                wh[:, half, :], OP.mult, OP.subtract)
                        elif n < NKB:
                            w = wpool.tile([128, OC], F16, tag="w",
                                           bufs=(NKB - nf8) * NCH,
                                           name=f"w{n}_{c}")
                            if n < 6:
                                nc.gpsimd.tensor_copy(w[:], st[:, i, :])
                            elif n < 10:
                                nc.scalar.copy(w[:], st[:, i, :])
                            else:
                                nc.vector.tensor_copy(w[:], st[:, i, :])
                            w_tiles[n][c] = w
                        else:
                            c0b = c0p.tile([128, OC], F16, tag="c0b",
                                           name=f"c0b{c}")
                            nc.scalar.copy(c0b[:], st[:, i, :])
                            c0bs[c] = c0b
                # emit next stream chain so its DVE work follows chunk-c casts
                if c == 0:
                    bases[1] = chain(1, xjs[1], frozenset())[0]
                elif c == 1:
                    bases[2] = chain(2, xjs[2], frozenset())[0]
                # matmuls: block j joins at chunk j, backfilling chunks < j
                for j in range(min(c + 1, NS)):
                    if c == j:
                        for cb in range(j + 1):
                            mm_chunk(accs[j], bases[j], cb,
                                     start=(cb == 0), stop=False)
                    else:
                        mm_chunk(accs[j], bases[j], c, start=False, stop=False)
                nc.tensor.matmul(pb[:], ones_col[:], c0bs[c][:],
                                 start=(c == 0), stop=(c == NCH - 1))
                if c in (2, 3, 4):
                    xjs[c + 1] = fetch_x(c + 1)

            def drain(j, acc, a8=None):
                ob = op_.tile([128, OC], F32, tag="ob", name=f"ob{j}")
                if a8 is None:
                    nc.vector.tensor_copy(ob[:], acc[:])
                else:
                    nc.vector.scalar_tensor_tensor(
                        ob[:], a8[:], 1.0 / WSC, acc[:], OP.mult, OP.add)
                nc.sync.dma_start(out_d[j * 128:(j + 1) * 128, :], ob[:])

            # ---- Bias row, then close + drain stream blocks ----
            nc.vector.tensor_copy(bias_h[:], pb[:])
            for j in range(NS):
                nc.tensor.matmul(accs[j][:], ones_row[:], bias_h[:],
                                 start=False, stop=True)
                drain(j, accs[j])

            # ---- Steady phase ----
            f8 = frozenset(FP8_SET)
            f16set = [n for n in range(NKB) if n not in f8]
            prev = None
            for j in range(NS, NBLK):
                xj = xjs.pop(j) if j in xjs else fetch_x(j)
                bas16, hi, lo = chain(j, xj, f8)
                acc = pacc.tile([128, OC], F32, tag="acc", name=f"acc{j}")
                first = True
                for n in f16set:
                    for c in range(NCH):
                        nc.tensor.matmul(
                            acc[:], bas16[n][:, c * 128:(c + 1) * 128],
                            w_tiles[n][c][:], start=first, stop=False)
                        first = False
                nc.tensor.matmul(acc[:], ones_row[:], bias_h[:],
                                 start=False, stop=True)
                a8 = None
                if f8:
                    a8 = pacc.tile([128, OC], F32, tag="acc", name=f"a8_{j}")
                    first = True
                    for n in sorted(f8):
                        hv = hi[n][:].rearrange("p (two f) -> p two f", f=128)
                        lv = lo[n][:].rearrange("p (two f) -> p two f", f=128)
                        for p2 in range(NCH // 2):
                            hT = hv[:, 2 * p2:2 * p2 + 2, :]
                            lT = lv[:, 2 * p2:2 * p2 + 2, :]
                            nc.tensor.matmul(a8[:], hT, wh_tiles[n][p2][:],
                                             start=first, stop=False,
                                             perf_mode=DR)
                            first = False
                            nc.tensor.matmul(a8[:], lT, wh_tiles[n][p2][:],
                                             start=False, stop=False,
                                             perf_mode=DR)
                            last = (n == max(f8) and p2 == NCH // 2 - 1)
                            nc.tensor.matmul(a8[:], hT, wl_tiles[n][p2][:],
                                             start=False, stop=last,
                                             perf_mode=DR)
                if prev is not None:
                    drain(*prev)
                prev = (j, acc, a8)
                if j + 2 < NBLK and (j + 2) not in xjs:
                    xjs[j + 2] = fetch_x(j + 2)
            drain(*prev)

    nc.compile()
    return nc


def _prep_core(x, coeffs, base_weight, core):
    b_idx, o_idx = divmod(core, MO)
    bsl = slice(b_idx * BC, (b_idx + 1) * BC)
    osl = slice(o_idx * OC, (o_idx + 1) * OC)
    xt = np.ascontiguousarray(x[bsl].T).reshape(NCH, 128, BC)
    ws = np.empty((NCH, NW, 128, OC), np.float32)
    for n in range(DEG):
        ws[:, n] = coeffs[:, osl, n + 1].reshape(NCH, 128, OC)
    ws[:, DEG] = base_weight[:, osl].reshape(NCH, 128, OC)
    ws[:, DEG + 1] = coeffs[:, osl, 0].reshape(NCH, 128, OC)
    return {"xt": xt, "ws": ws}


def kernel(x, coeffs, base_weight):
    global LAST_RESULTS
    assert x.shape == (B, IN) and coeffs.shape == (IN, OUT, DEG + 1)
    assert base_weight.shape == (IN, OUT)

    if "nc" not in _CACHE:
        _CACHE["nc"] = _build_nc()
    nc = _CACHE["nc"]

    x = np.ascontiguousarray(x, dtype=np.float32)
    coeffs = np.ascontiguousarray(coeffs, dtype=np.float32)
    base_weight = np.ascontiguousarray(base_weight, dtype=np.float32)

    in_maps = [_prep_core(x, coeffs, base_weight, core) for core in range(8)]

    res = run_bass_kernel_spmd(nc, in_maps, core_ids=list(range(8)))
    LAST_RESULTS = res

    out = np.empty((B, OUT), dtype=np.float32)
    for core in range(8):
        b_idx, o_idx = divmod(core, MO)
        out[b_idx * BC:(b_idx + 1) * BC, o_idx * OC:(o_idx + 1) * OC] = \
            res.results[core]["out"]
    return out
